# revision 5
# baseline (speedup 1.0000x reference)
"""Trainium2 Bass kernel for nn_ContrastiveLoss2 (SimCLR-style NT-Xent loss).

Math (matches the jax reference):
    z  = concat([z_augment, z_orig])                       # [N=8192, D=256]
    zn = z / max(||z||, eps)                               # row L2 normalize
    S  = zn @ zn.T                                         # cosine sim [N, N]
    loss_i = -S[i, i+-B]/tau + log( sum_{j != i} exp(S[i,j]/tau) )
    out = mean_i loss_i                                    # tau = 0.5

Key identity used: the positive logit appears exactly once in the softmax
denominator whether it is indexed as "pos" or as a "negative", so the
denominator is simply the full row sum of exp(S/tau) minus the diagonal
term exp(S_ii/tau).

Distribution: data-parallel over the 8192 rows -> 1024 rows per core.
Each core receives the full z ROTATED so that its own rows sit at
[0:1024) and the positive partners at [4096:5120).  The program is
identical on every core; only the data differs (pure SPMD, no
collectives).  The host sums the 8 per-core partial losses.

Per-core pipeline:
  - DMA z (8 MB fp32) in 4 group chunks of 2 MB
  - row sumsq (DVE) -> 1/norm = exp(-0.5*ln(sumsq)) (ACT, one table set)
  - zn = z * invnorm cast to bf16 (DVE), transposed via PE into znT
  - S row-block = znT_own.T @ znT (bf16 matmul, fp32 PSUM accumulate)
  - exp(2*S) + row-sum fused on ACT (scale=2.0, accum_out)
  - loss assembly (DVE/ACT) -> [128, 8] per-row losses -> DRAM
"""

import sys

import numpy as np

try:
    import concourse  # noqa: F401
except ImportError:  # pragma: no cover
    sys.path.insert(0, "/opt/trn_rl_repo")

N_CORES = 8
N = 8192          # total rows (2B)
D = 256           # feature dim
B = 4096          # batch (positive offset)
ROWS_PER_CORE = N // N_CORES   # 1024
P = 128           # SBUF partitions
NT = N // P       # 64 natural row-tiles
NG = 4            # transpose/matmul column groups
GT = NT // NG     # 16 tiles per group
NI = ROWS_PER_CORE // P        # 8 own row-tiles
CH = 512          # matmul chunk (one PSUM bank of fp32)
GW = 2048         # ACT group width = 4 chunks
TAU = 0.5


def _kernel_body(ctx, tc, out_ap, zr_ap):
    import concourse.bass as bass
    from concourse import mybir
    from concourse.masks import make_identity

    nc = tc.nc
    f32 = mybir.dt.float32
    bf16 = mybir.dt.bfloat16
    Fn = mybir.ActivationFunctionType
    Op = mybir.AluOpType

    p_const = ctx.enter_context(tc.tile_pool(name="const", bufs=1))
    p_znat = ctx.enter_context(tc.tile_pool(name="znat", bufs=1))
    p_zbf = ctx.enter_context(tc.tile_pool(name="zbf", bufs=1))
    p_znt = ctx.enter_context(tc.tile_pool(name="znt", bufs=1))
    p_stats = ctx.enter_context(tc.tile_pool(name="stats", bufs=1))
    p_tmp = ctx.enter_context(tc.tile_pool(name="tmp", bufs=3))
    p_exp = ctx.enter_context(tc.tile_pool(name="exp", bufs=3))
    p_ps = ctx.enter_context(tc.tile_pool(name="ps", bufs=2, space="PSUM"))

    ident = p_const.tile([P, P], bf16)
    make_identity(nc, ident[:])

    znat = p_znat.tile([P, NT * D], f32)          # raw rows, natural layout
    zbf = p_zbf.tile([P, NT * D], bf16)           # normalized rows, bf16
    znt = [p_znt.tile([P, 2, GW], bf16, tag=f"znt{g}", name=f"znt{g}")
           for g in range(NG)]
    ss = p_stats.tile([P, NT], f32, tag="ss")     # per-row sum of squares
    inv = p_stats.tile([P, NT], f32, tag="inv")   # per-row 1/norm
    posr = p_stats.tile([P, NI], f32, tag="posr")  # raw dot(z_i, z_partner)
    sums = p_stats.tile([P, NI * NG], f32, tag="sums")  # exp row-sum parts

    for g in range(NG):
        # ---- load 2 MB of rows: rotated rows [g*2048, (g+1)*2048) ----
        nc.sync.dma_start(
            out=znat[:, g * GT * D:(g + 1) * GT * D].rearrange(
                "p (t c) -> p t c", c=D),
            in_=zr_ap[g * GT * P:(g + 1) * GT * P, :].rearrange(
                "(t p) c -> p t c", p=P),
        )

        # ---- per-tile row sum of squares (DVE; TTR is broken on HW here) ----
        for t in range(GT):
            ti = g * GT + t
            zt = znat[:, ti * D:(ti + 1) * D]
            sq = p_tmp.tile([P, D], f32, tag="sq")
            nc.vector.tensor_mul(sq[:], zt, zt)
            nc.vector.tensor_reduce(
                ss[:, ti:ti + 1], sq[:], axis=mybir.AxisListType.X, op=Op.add)

        # ---- 1/norm = exp(-0.5 * ln(sumsq)); single ACT table set ----
        lns = p_tmp.tile([P, GT], f32, tag="lns")
        nc.scalar.activation(lns[:], ss[:, g * GT:(g + 1) * GT], Fn.Ln)
        nc.scalar.activation(
            inv[:, g * GT:(g + 1) * GT], lns[:], Fn.Exp, scale=-0.5)

        # ---- normalize to bf16 + transpose into znT (PE + DVE) ----
        for t in range(GT):
            ti = g * GT + t
            zt = znat[:, ti * D:(ti + 1) * D]
            zb = zbf[:, ti * D:(ti + 1) * D]
            nc.vector.tensor_scalar_mul(zb, zt, inv[:, ti:ti + 1])
            ps = p_ps.tile([P, 2 * P], bf16, tag="ps")
            nc.tensor.transpose(ps[:, 0:P], zb[:, 0:P], ident[:])
            nc.tensor.transpose(ps[:, P:2 * P], zb[:, P:2 * P], ident[:])
            nc.vector.tensor_copy(
                out=znt[g][:, :, t * P:(t + 1) * P],
                in_=ps[:].rearrange("p (k j) -> p k j", k=2),
            )

        if g == 2:
            # raw positive dots need own tiles (g0) and partner tiles (g2)
            for i in range(NI):
                sq = p_tmp.tile([P, D], f32, tag="sq")
                nc.vector.tensor_mul(
                    sq[:],
                    znat[:, i * D:(i + 1) * D],
                    znat[:, (GT * 2 + i) * D:(GT * 2 + i + 1) * D])
                nc.vector.tensor_reduce(
                    posr[:, i:i + 1], sq[:],
                    axis=mybir.AxisListType.X, op=Op.add)

        # ---- S row-blocks against this column group + fused exp/rowsum ----
        for i in range(NI):
            ps = p_ps.tile([P, GW], f32, tag="ps")
            for jc in range(GW // CH):
                for k in range(2):
                    nc.tensor.matmul(
                        ps[:, jc * CH:(jc + 1) * CH],
                        lhsT=znt[0][:, k, i * P:(i + 1) * P],
                        rhs=znt[g][:, k, jc * CH:(jc + 1) * CH],
                        start=(k == 0), stop=(k == 1),
                    )
            ex = p_exp.tile([P, GW], bf16, tag="ex")
            nc.scalar.activation(
                ex[:], ps[:], Fn.Exp, scale=2.0,
                accum_out=sums[:, i * NG + g:i * NG + g + 1],
            )

    # ---- loss assembly: loss = -2*pos + ln(rowsum - exp(2*selfsim)) ----
    totals = p_stats.tile([P, NI], f32, tag="tot")
    nc.vector.tensor_reduce(
        totals[:], sums[:].rearrange("p (i g) -> p i g", g=NG),
        axis=mybir.AxisListType.X, op=Op.add,
    )
    s1 = p_stats.tile([P, NI], f32, tag="s1")
    nc.vector.tensor_tensor(s1[:], ss[:, 0:NI], inv[:, 0:NI], op=Op.mult)
    s2 = p_stats.tile([P, NI], f32, tag="s2")
    nc.vector.tensor_tensor(s2[:], s1[:], inv[:, 0:NI], op=Op.mult)
    es = p_stats.tile([P, NI], f32, tag="es")
    nc.scalar.activation(es[:], s2[:], Fn.Exp, scale=2.0)
    neg = p_stats.tile([P, NI], f32, tag="neg")
    nc.vector.tensor_sub(neg[:], totals[:], es[:])
    lg = p_stats.tile([P, NI], f32, tag="lg")
    nc.scalar.activation(lg[:], neg[:], Fn.Ln)
    p1 = p_stats.tile([P, NI], f32, tag="p1")
    nc.vector.tensor_tensor(p1[:], posr[:], inv[:, 0:NI], op=Op.mult)
    p2 = p_stats.tile([P, NI], f32, tag="p2")
    nc.vector.tensor_tensor(
        p2[:], p1[:], inv[:, GT * 2:GT * 2 + NI], op=Op.mult)
    loss = p_stats.tile([P, NI], f32, tag="loss")
    nc.vector.scalar_tensor_tensor(
        out=loss[:], in0=p2[:], scalar=-2.0 / (2.0 * TAU), in1=lg[:],
        op0=Op.mult, op1=Op.add,
    )
    nc.sync.dma_start(out=out_ap, in_=loss[:])


def build_nc():
    """Build (once) the Bass module shared by all 8 cores."""
    from contextlib import ExitStack

    from concourse import bacc, mybir
    import concourse.tile as tile

    nc = bacc.Bacc("TRN2", target_bir_lowering=False, debug=False)
    zr = nc.dram_tensor("zr", [N, D], mybir.dt.float32,
                        kind="ExternalInput").ap()
    out = nc.dram_tensor("out", [P, NI], mybir.dt.float32,
                         kind="ExternalOutput").ap()
    with tile.TileContext(nc) as tc:
        with ExitStack() as ctx:
            _kernel_body(ctx, tc, out, zr)
    return nc


_NC = None


def _get_nc(finalized=True):
    global _NC
    if _NC is None:
        _NC = build_nc()
    if finalized and not _NC.is_finalized():
        _NC.finalize()
    return _NC


def make_in_maps(z_orig, z_augment):
    z = np.ascontiguousarray(
        np.concatenate([np.asarray(z_augment, dtype=np.float32),
                        np.asarray(z_orig, dtype=np.float32)], axis=0))
    return [{"zr": np.roll(z, -ROWS_PER_CORE * c, axis=0)}
            for c in range(N_CORES)]


def reduce_outputs(results):
    total = 0.0
    for r in results:
        total += float(np.asarray(r["out"], dtype=np.float64).sum())
    return np.float32(total / N)


def kernel(z_orig, z_augment):
    from concourse.bass_utils import run_bass_kernel_spmd

    nc = _get_nc()
    in_maps = make_in_maps(z_orig, z_augment)
    res = run_bass_kernel_spmd(nc, in_maps, core_ids=list(range(N_CORES)))
    return reduce_outputs(res.results)


# revision 16
# speedup vs baseline: 512.6484x; 512.6484x over previous
"""Trainium2 Bass kernel for nn_ContrastiveLoss2 (SimCLR-style NT-Xent loss).

Math (matches the jax reference):
    z  = concat([z_augment, z_orig])                       # [N=8192, D=256]
    zn = z / max(||z||, eps)                               # row L2 normalize
    S  = zn @ zn.T                                         # cosine sim [N, N]
    loss_i = -S[i, i+-B]/tau + log( sum_{j != i} exp(S[i,j]/tau) )
    out = mean_i loss_i                                    # tau = 0.5

Key identity used: the positive logit appears exactly once in the softmax
denominator whether it is indexed as "pos" or as a "negative", so the
denominator is simply the full row sum of exp(S/tau) minus the diagonal
term exp(S_ii/tau).

Distribution: data-parallel over the 8192 rows -> 1024 rows per core.
Each core receives the full z ROTATED so that its own rows sit at
[0:1024) and the positive partners at [4096:5120).  The program is
identical on every core; only the data differs (pure SPMD, no
collectives).  The host sums the 8 per-core partial losses.

Per-core pipeline:
  - DMA z (8 MB fp32) in 4 group chunks of 2 MB
  - row sumsq (DVE) -> 1/norm = exp(-0.5*ln(sumsq)) (ACT, one table set)
  - zn = z * invnorm cast to bf16 (DVE), transposed via PE into znT
  - S row-block = znT_own.T @ znT (bf16 matmul, fp32 PSUM accumulate)
  - exp(2*S) + row-sum fused on ACT (scale=2.0, accum_out)
  - loss assembly (DVE/ACT) -> [128, 8] per-row losses -> DRAM
"""

import sys

import numpy as np

try:
    import concourse  # noqa: F401
except ImportError:  # pragma: no cover
    sys.path.insert(0, "/opt/trn_rl_repo")

N_CORES = 8
N = 8192          # total rows (2B)
D = 256           # feature dim
B = 4096          # batch (positive offset)
ROWS_PER_CORE = N // N_CORES   # 1024
P = 128           # SBUF partitions
NT = N // P       # 64 natural row-tiles
NG = 4            # transpose/matmul column groups
GT = NT // NG     # 16 tiles per group
NI = ROWS_PER_CORE // P        # 8 own row-tiles
CH = 512          # matmul chunk (one PSUM bank of fp32)
GW = 2048         # ACT group width = 4 chunks
TAU = 0.5


def _kernel_body(ctx, tc, out_ap, zr_ap):
    import concourse.bass as bass
    from concourse import mybir
    from concourse.masks import make_identity

    nc = tc.nc
    f32 = mybir.dt.float32
    bf16 = mybir.dt.bfloat16
    Fn = mybir.ActivationFunctionType
    Op = mybir.AluOpType

    p_const = ctx.enter_context(tc.tile_pool(name="const", bufs=1))
    p_znat = ctx.enter_context(tc.tile_pool(name="znat", bufs=1))
    p_zbf = ctx.enter_context(tc.tile_pool(name="zbf", bufs=1))
    p_znt = ctx.enter_context(tc.tile_pool(name="znt", bufs=1))
    p_stats = ctx.enter_context(tc.tile_pool(name="stats", bufs=1))
    p_tmp = ctx.enter_context(tc.tile_pool(name="tmp", bufs=3))
    p_exp = ctx.enter_context(tc.tile_pool(name="exp", bufs=3))
    p_ps = ctx.enter_context(tc.tile_pool(name="ps", bufs=2, space="PSUM"))
    p_dram = ctx.enter_context(tc.tile_pool(name="dram", bufs=1, space="DRAM"))
    p_zntbf = ctx.enter_context(tc.tile_pool(name="zntbf", bufs=1))

    fp8 = mybir.dt.float8e4
    znat = p_znat.tile([P, NT * D], f32)          # raw rows, natural layout
    zbf = p_zbf.tile([P, NT * D], bf16)           # normalized rows, bf16
    # znT in fp8e4, laid out [ki=128, ko=2, row] for DoubleRow (K=256/pass)
    znt = [p_znt.tile([P, 2, GW], fp8, tag=f"znt{g}", name=f"znt{g}")
           for g in range(NG)]
    znt_bf = [p_zntbf.tile([P, 2, GW], bf16, tag=f"zntbf{g}", name=f"zntbf{g}")
              for g in range(NG)]
    zbounce = p_dram.tile([N, D], bf16)           # DRAM bounce for transpose
    ss = p_stats.tile([P, NT], f32, tag="ss")     # per-row sum of squares
    inv = p_stats.tile([P, NT], f32, tag="inv")   # per-row 1/norm
    posr = p_stats.tile([P, NI], f32, tag="posr")  # raw dot(z_i, z_partner)
    sums = p_stats.tile([P, NI * NG], f32, tag="sums")  # exp row-sum parts

    def stage_a(g):
        # ---- load 2 MB of rows: rotated rows [g*2048, (g+1)*2048) ----
        nc.sync.dma_start(
            out=znat[:, g * GT * D:(g + 1) * GT * D].rearrange(
                "p (t c) -> p t c", c=D),
            in_=zr_ap[g * GT * P:(g + 1) * GT * P, :].rearrange(
                "(t p) c -> p t c", p=P),
        )

        # ---- batched row sum of squares for the whole group (DVE) ----
        zg = znat[:, g * GT * D:(g + 1) * GT * D]
        sq = p_tmp.tile([P, GT * D], f32, tag="sq")
        nc.vector.tensor_mul(sq[:], zg, zg)
        nc.vector.tensor_reduce(
            ss[:, g * GT:(g + 1) * GT],
            sq[:].rearrange("p (t c) -> p t c", c=D),
            axis=mybir.AxisListType.X, op=Op.add)

        # ---- 1/norm = exp(-0.5 * ln(sumsq)); single ACT table set ----
        lns = p_tmp.tile([P, GT], f32, tag="lns")
        nc.scalar.activation(lns[:], ss[:, g * GT:(g + 1) * GT], Fn.Ln)
        nc.scalar.activation(
            inv[:, g * GT:(g + 1) * GT], lns[:], Fn.Exp, scale=-0.5)

        # ---- normalize whole group to bf16 in ONE DVE op (broadcast inv) ----
        inv_bc = inv[:, g * GT:(g + 1) * GT].rearrange(
            "p (t o) -> p t o", o=1).broadcast_to((P, GT, D))
        nc.vector.tensor_mul(
            zbf[:, g * GT * D:(g + 1) * GT * D].rearrange(
                "p (t c) -> p t c", c=D),
            zg.rearrange("p (t c) -> p t c", c=D), inv_bc)

        # ---- transpose via DMA xbar: SBUF->DRAM bounce, then 2 transposed
        # ---- loads [2048,128]->[128,2048], then one DVE cast to fp8 ----
        nc.sync.dma_start(
            out=zbounce[g * GT * P:(g + 1) * GT * P, :].rearrange(
                "(t p) c -> p t c", p=P),
            in_=zbf[:, g * GT * D:(g + 1) * GT * D].rearrange(
                "p (t c) -> p t c", c=D),
        )
        for k in range(2):
            nc.sync.dma_start_transpose(
                znt_bf[g][:, k, :],
                zbounce[g * GT * P:(g + 1) * GT * P, k * P:(k + 1) * P],
            )
        nc.vector.tensor_copy(out=znt[g][:], in_=znt_bf[g][:])

        if g == 2:
            # raw positive dots need own tiles (g0) and partner tiles (g2)
            for i in range(NI):
                sq = p_tmp.tile([P, D], f32, tag="sq")
                nc.vector.tensor_mul(
                    sq[:],
                    znat[:, i * D:(i + 1) * D],
                    znat[:, (GT * 2 + i) * D:(GT * 2 + i + 1) * D])
                nc.vector.tensor_reduce(
                    posr[:, i:i + 1], sq[:],
                    axis=mybir.AxisListType.X, op=Op.add)

    def stage_b(g):
        # ---- S row-blocks against this column group + fused exp/rowsum ----
        for i in range(NI):
            ps = p_ps.tile([P, GW], f32, tag="ps", name="psmm")
            for jc in range(GW // CH):
                nc.tensor.matmul(
                    ps[:, jc * CH:(jc + 1) * CH],
                    lhsT=znt[0][:, :, i * P:(i + 1) * P],
                    rhs=znt[g][:, :, jc * CH:(jc + 1) * CH],
                    start=True, stop=True,
                    perf_mode=mybir.MatmulPerfMode.DoubleRow,
                )
            ex = p_exp.tile([P, GW], bf16, tag="ex", name="ex")
            nc.scalar.activation(
                ex[:], ps[:], Fn.Exp, scale=2.0,
                accum_out=sums[:, i * NG + g:i * NG + g + 1],
            )

    # software-pipeline by one group: transposes of g+1 are emitted before
    # the matmul/exp block of g, so PE races ahead while ACT drains exps.
    stage_a(0)
    for g in range(1, NG):
        stage_a(g)
        stage_b(g - 1)
    stage_b(NG - 1)

    # ---- loss assembly: loss = -2*pos + ln(rowsum - exp(2*selfsim)) ----
    totals = p_stats.tile([P, NI], f32, tag="tot")
    nc.vector.tensor_reduce(
        totals[:], sums[:].rearrange("p (i g) -> p i g", g=NG),
        axis=mybir.AxisListType.X, op=Op.add,
    )
    s1 = p_stats.tile([P, NI], f32, tag="s1")
    nc.vector.tensor_tensor(s1[:], ss[:, 0:NI], inv[:, 0:NI], op=Op.mult)
    s2 = p_stats.tile([P, NI], f32, tag="s2")
    nc.vector.tensor_tensor(s2[:], s1[:], inv[:, 0:NI], op=Op.mult)
    es = p_stats.tile([P, NI], f32, tag="es")
    nc.scalar.activation(es[:], s2[:], Fn.Exp, scale=2.0)
    neg = p_stats.tile([P, NI], f32, tag="neg")
    nc.vector.tensor_sub(neg[:], totals[:], es[:])
    lg = p_stats.tile([P, NI], f32, tag="lg")
    nc.scalar.activation(lg[:], neg[:], Fn.Ln)
    p1 = p_stats.tile([P, NI], f32, tag="p1")
    nc.vector.tensor_tensor(p1[:], posr[:], inv[:, 0:NI], op=Op.mult)
    p2 = p_stats.tile([P, NI], f32, tag="p2")
    nc.vector.tensor_tensor(
        p2[:], p1[:], inv[:, GT * 2:GT * 2 + NI], op=Op.mult)
    loss = p_stats.tile([P, NI], f32, tag="loss")
    nc.vector.scalar_tensor_tensor(
        out=loss[:], in0=p2[:], scalar=-2.0 / (2.0 * TAU), in1=lg[:],
        op0=Op.mult, op1=Op.add,
    )
    nc.sync.dma_start(out=out_ap, in_=loss[:])


def build_nc():
    """Build (once) the Bass module shared by all 8 cores."""
    from contextlib import ExitStack

    from concourse import bacc, mybir
    import concourse.tile as tile

    nc = bacc.Bacc("TRN2", target_bir_lowering=False, debug=False)
    zr = nc.dram_tensor("zr", [N, D], mybir.dt.float32,
                        kind="ExternalInput").ap()
    out = nc.dram_tensor("out", [P, NI], mybir.dt.float32,
                         kind="ExternalOutput").ap()
    with tile.TileContext(nc) as tc:
        with ExitStack() as ctx:
            _kernel_body(ctx, tc, out, zr)
    return nc


_NC = None


def _get_nc(finalized=True):
    global _NC
    if _NC is None:
        _NC = build_nc()
    if finalized and not _NC.is_finalized():
        _NC.finalize()
    return _NC


def make_in_maps(z_orig, z_augment):
    z = np.ascontiguousarray(
        np.concatenate([np.asarray(z_augment, dtype=np.float32),
                        np.asarray(z_orig, dtype=np.float32)], axis=0))
    return [{"zr": np.roll(z, -ROWS_PER_CORE * c, axis=0)}
            for c in range(N_CORES)]


def reduce_outputs(results):
    total = 0.0
    for r in results:
        total += float(np.asarray(r["out"], dtype=np.float64).sum())
    return np.float32(total / N)


def kernel(z_orig, z_augment):
    from concourse.bass_utils import run_bass_kernel_spmd

    nc = _get_nc()
    in_maps = make_in_maps(z_orig, z_augment)
    res = run_bass_kernel_spmd(nc, in_maps, core_ids=list(range(N_CORES)))
    return reduce_outputs(res.results)
